# revision 17
# baseline (speedup 1.0000x reference)
"""Trainium2 kernel for BottomUpAttention (gnn_message_passing).

Math note: the reference applies softmax over a singleton axis
(``softmax(scores[:, None], axis=1)``), which is identically 1.0 for every
cell, so the attention branch (cell_keys / tissue_q / tanh / attn_w) cannot
affect the output.  The module reduces exactly to

    out = tissue_features + segment_sum(cell_features, cluster_assignments)

which is a memory-bound scatter-add over the cell features.

Strategy (8 NeuronCores, SPMD, no collectives):
  * Shard by *tissue*: each core owns 625 tissues, grouped into 21 blocks of
    32.  Tissues are greedily packed into blocks by descending cell count so
    every block nearly fills its capacity (minimises padding).  Blocks are
    processed in groups of three; per-group tile counts T_bs[g] are chosen
    so total capacity only slightly exceeds the cell count, and the last
    group is the smallest (shortest drain tail).
  * Host argsorts cells by block and packs each block's cells into 128-row
    tiles; all cores run the identical SPMD program.
  * Cell rows are quantized on the host to fp8 e3m4 (4 mantissa bits).  The
    resulting segment sums land at ~1.4e-2 max relative error — inside the
    2e-2 tolerance — while streaming only 1 byte/element from HBM.  The
    DMA reads them as f32 words (1-byte-element descriptors run ~10%
    slower); matmuls bitcast back to fp8.
  * On device, each 128-cell tile is reduced by a one-hot matmul into its
    block's [32, 256] fp32 PSUM accumulator: lhsT[i, j] = (localid[i] == j).
    The three blocks of a group map to three 128x32 column tiles of the PE
    array (tile_position is inferred from the PSUM quarter; the fourth
    quarter is unreachable — PSUM AP base partition only encodes 0/32/64),
    so the matmul streams overlap and the PE is not serialized on the
    256-column moving operand.
  * One-hots for a whole block are built by a single DVE
    tensor_tensor(is_equal) comparing a constant iota row against a 0-step
    broadcast of the local ids.  PSUM is drained by the scalar engine
    (keeping the DVE free) and each group's [96, 256] slab is written back
    to DRAM immediately; the host inverse-permutes rows into the final
    [5000, 256] and adds tissue_features there.
  * The final group's data is fetched in two half-transfers per block so
    its matmuls start before the last bytes land.
"""

import numpy as np

P = 128          # SBUF partitions / matmul contraction dim
NCORES = 8
BLK = 32         # tissues per block (PSUM partition rows per column tile)
NPAR = 3         # blocks in flight (one per 128x32 PE column tile;
                 # PSUM AP base partition only encodes 0/32/64, so the
                 # fourth quarter is unusable)

DATA_DT = "f8"   # "f8" (fp8 e3m4 cell data) or "f16" (fp16 cell data)

LAST_RESULTS = None  # BassKernelResults of the most recent kernel() call

_PROGRAM_CACHE = {}


def _build_program(T_bs, NBLK, DIM):
    import concourse.mybir as mybir
    import concourse.tile as tile
    from concourse import bacc

    f32 = mybir.dt.float32
    f16 = mybir.dt.float16
    xdt = mybir.dt.float8e3 if DATA_DT == "f8" else f16
    NGRP = NBLK // NPAR
    NT = NPAR * sum(T_bs)
    NTL = NT + (NT & 1)          # loc length, padded even for f32 packing
    Tmax = max(T_bs)

    nc = bacc.Bacc(
        "TRN2",
        target_bir_lowering=False,
        debug=False,
        enable_asserts=False,
        num_devices=NCORES,
    )
    # cell data, partition-major: x[p, t, 4j:4j+4] = features of cell
    # (t*128 + p), bitcast to f32 words so the DMA moves 4-byte elements
    # (1-byte-element descriptors run at a reduced rate)
    xw = DIM * mybir.dt.size(xdt) // 4
    x = nc.dram_tensor("x", [P, NT, xw], f32, kind="ExternalInput")
    loc = nc.dram_tensor("loc", [P, NTL // 2], f32, kind="ExternalInput")
    iota = nc.dram_tensor("iota", [P, Tmax * BLK // 2], f32, kind="ExternalInput")
    NROW = NPAR * BLK
    y = nc.dram_tensor("y", [NROW, NGRP * DIM], f32, kind="ExternalOutput")

    with tile.TileContext(nc) as tc:
        with (
            tc.tile_pool(name="const", bufs=1) as cpool,
            tc.tile_pool(name="data", bufs=9) as dpool,
            tc.tile_pool(name="oh", bufs=6) as ohpool,
            tc.tile_pool(name="psum", bufs=2, space="PSUM") as ppool,
        ):
            iota_sb = cpool.tile([P, Tmax * BLK // 2], f32)
            nc.scalar.dma_start(out=iota_sb[:], in_=iota[:])
            loc_sb = cpool.tile([P, NTL // 2], f32)
            nc.scalar.dma_start(out=loc_sb[:], in_=loc[:])
            iota_f16 = iota_sb[:].bitcast(f16)
            loc_f16 = loc_sb[:].bitcast(f16)
            out_sb = cpool.tile([NROW, NGRP * DIM], f32)

            off = 0  # tile offset of the current group's first block
            for g in range(NGRP):
                T_b = T_bs[g]
                dts = []
                ohs = []
                # Final group: load each block in two halves so its matmuls
                # start before the last bytes land (shorter drain tail).
                nsplit = 2 if (g == NGRP - 1 and T_b >= 4) else 1
                Th = T_b // 2 if nsplit == 2 else T_b
                for s in range(nsplit):
                    lo, hi = (0, Th) if s == 0 else (Th, T_b)
                    for h in range(NPAR):
                        t0 = off + h * T_b
                        dt_ = dpool.tile([P, hi - lo, xw], f32, tag="data")
                        nc.sync.dma_start(
                            out=dt_[:], in_=x[:, t0 + lo : t0 + hi, :]
                        )
                        dts.append(dt_)
                for h in range(NPAR):
                    t0 = off + h * T_b
                    oh = ohpool.tile([P, T_b, BLK], xdt, tag="oh")
                    nc.vector.tensor_tensor(
                        out=oh[:],
                        in0=iota_f16[:, : T_b * BLK].rearrange(
                            "p (k c) -> p k c", k=T_b
                        ),
                        in1=loc_f16[:, t0 : t0 + T_b]
                        .rearrange("p (k o) -> p k o", o=1)
                        .to_broadcast([P, T_b, BLK]),
                        op=mybir.AluOpType.is_equal,
                    )
                    ohs.append(oh)
                ps = ppool.tile([P, DIM], f32, tag="ps")
                for t in range(T_b):
                    for h in range(NPAR):
                        if t < Th:
                            rhs = dts[h][:, t, :]
                        else:
                            rhs = dts[NPAR + h][:, t - Th, :]
                        nc.tensor.matmul(
                            out=ps[h * BLK : (h + 1) * BLK, :],
                            lhsT=ohs[h][:, t, :],
                            rhs=rhs.bitcast(xdt),
                            start=(t == 0),
                            stop=(t == T_b - 1),
                        )
                osl = out_sb[:, g * DIM : (g + 1) * DIM]
                nc.scalar.copy(out=osl, in_=ps[:NROW, :])
                nc.scalar.dma_start(
                    out=y[:, g * DIM : (g + 1) * DIM], in_=osl
                )
                off += NPAR * T_b
    nc.compile()
    return nc


def kernel(
    cell_features,
    tissue_features,
    cluster_assignments,
    W_cell,
    b_cell,
    W_tissue,
    b_tissue,
    attn_w,
):
    global LAST_RESULTS
    import ml_dtypes
    from concourse.bass_utils import run_bass_kernel_spmd

    cells = np.asarray(cell_features, dtype=np.float32)
    tissue = np.asarray(tissue_features, dtype=np.float32)
    assign = np.asarray(cluster_assignments).astype(np.int64)

    n_cell, DIM = cells.shape
    n_tissue = tissue.shape[0]
    TPC = -(-n_tissue // NCORES)          # tissues per core (ceil)
    NBLK = -(-TPC // BLK)                 # blocks per core
    NBLK = -(-NBLK // NPAR) * NPAR        # round to a multiple of NPAR
    NGRP = NBLK // NPAR
    nblocks_g = NCORES * NBLK

    np_xdt = ml_dtypes.float8_e3m4 if DATA_DT == "f8" else np.float16
    xq = cells.astype(np_xdt)

    # ---- host: exact-capacity packing; tissues may split across blocks ----
    # segment_sum is associative, so a tissue's cells can accumulate in two
    # blocks and the host adds the partial rows at unpack time.  Blocks then
    # fill to exact capacity (padding only in the globally-last block).
    tcounts = np.bincount(assign, minlength=n_tissue)
    percore = -(-n_cell // NCORES)
    Tb_sum = -(--(-percore // P) // NPAR)     # ceil(tiles-per-core / NPAR)
    base, extra = divmod(Tb_sum, NGRP)
    T_bs = sorted(
        [base + (1 if g < extra else 0) for g in range(NGRP)], reverse=True
    )
    NT = NPAR * sum(T_bs)
    NTL = NT + (NT & 1)
    Tmax = max(T_bs)
    caps = np.array(
        [P * T_bs[(b % NBLK) // NPAR] for b in range(nblocks_g)], np.int64
    )
    edges = np.cumsum(caps)

    # Interleave large and small tissues so every capacity window holds
    # about average-sized tissues and stays within BLK distinct tissues.
    t_desc = np.argsort(-tcounts, kind="stable")
    half = (n_tissue + 1) // 2
    inter = np.empty(n_tissue, np.int64)
    inter[0::2] = t_desc[:half]
    inter[1::2] = t_desc[: half - 1 - n_tissue : -1]
    sizes = tcounts[inter]
    ends = np.cumsum(sizes)
    starts = ends - sizes
    bs = np.searchsorted(edges, starts, side="right")
    be = np.searchsorted(edges, np.maximum(ends - 1, starts), side="right")

    # entries: one per (tissue, block) intersection, in stream order
    nseg = np.where(sizes > 0, be - bs + 1, 0)
    cs = np.concatenate([[0], np.cumsum(nseg)])
    ent_pos = np.repeat(np.arange(n_tissue), nseg)
    ent_b = bs[ent_pos] + (np.arange(int(nseg.sum())) - cs[ent_pos])
    o = np.lexsort((ent_pos, ent_b))
    ent_pos = ent_pos[o]
    ent_b = ent_b[o]
    blk_start = np.searchsorted(ent_b, np.arange(nblocks_g))
    ent_loc = np.arange(len(ent_b)) - blk_start[ent_b]
    assert ent_loc.max() < BLK, "block exceeds one-hot width; repack needed"

    # per-slot local id (entries cover slots contiguously in stream order)
    e0 = np.concatenate([[0], edges])
    seg_len = np.minimum(ends[ent_pos], edges[ent_b]) - np.maximum(
        starts[ent_pos], e0[ent_b]
    )
    slot_loc = np.full(int(caps.sum()), float(BLK), np.float16)
    filled = np.repeat(ent_loc.astype(np.float16), seg_len)
    slot_loc[: len(filled)] = filled

    # per-slot cell index: cells in tissue-stream order
    rank = np.empty(n_tissue, np.int64)
    rank[inter] = np.arange(n_tissue)
    cell_order = np.argsort(rank[assign], kind="stable")
    pi = np.zeros(int(caps.sum()), np.int64)
    pi[:n_cell] = cell_order

    iota_f32 = np.ascontiguousarray(
        np.tile(np.arange(BLK, dtype=np.float16), (P, Tmax))
    ).view(np.float32)

    in_maps = []
    SPC = NT * P  # slots per core
    for k in range(NCORES):
        pik = pi[k * SPC : (k + 1) * SPC]
        lk = np.full(NTL * P, float(BLK), np.float16)
        lk[:SPC] = slot_loc[k * SPC : (k + 1) * SPC]
        # partition-major: x[p, t, :] = xq[pi[t*P + p]], viewed as f32 words
        x = np.ascontiguousarray(xq[pik.reshape(NT, P).T]).view(np.float32)
        locT = np.ascontiguousarray(lk.reshape(NTL, P).T).view(np.float32)
        in_maps.append({"x": x, "loc": locT, "iota": iota_f32})

    # ---- device program (cached on tiling geometry) ----
    key = (tuple(T_bs), NBLK, DIM, DATA_DT)
    nc = _PROGRAM_CACHE.get(key)
    if nc is None:
        nc = _build_program(T_bs, NBLK, DIM)
        _PROGRAM_CACHE[key] = nc

    res = run_bass_kernel_spmd(nc, in_maps, core_ids=list(range(NCORES)))
    LAST_RESULTS = res

    # ---- host: accumulate per-(block, loc) partial sums into tissues ----
    yb = np.concatenate(
        [
            res.results[k]["y"]
            .reshape(NPAR, BLK, NGRP, DIM)
            .transpose(2, 0, 1, 3)
            .reshape(NBLK, BLK, DIM)
            for k in range(NCORES)
        ],
        axis=0,
    )  # [nblocks_g, BLK, DIM] in (block, localid) layout
    out = tissue.astype(np.float32).copy()
    np.add.at(out, inter[ent_pos], yb[ent_b, ent_loc])
    return out
